# revision 8
# baseline (speedup 1.0000x reference)
"""GQA causal attention (llama3-style RoPE) on 8 TRN2 NeuronCores.

Sharding: tensor-parallel over heads. Core c gets q-heads 4c..4c+3 and
kv-head c (GQA groups intact), plus the matching row-block of wo.T.
Each core computes a full [S, D] partial of the output projection;
the host sums the 8 partials (the "all-reduce" of the row-sharded wo).

Per-core pipeline (all layouts chosen so no on-device transpose of x/q/k
is ever needed):
  qkvT[col, s]  = wqkvT.T @ xT          (weights stationary, xT streaming)
  RoPE on qT/kT (de-interleaved pair layout via host-permuted weight rows)
  sT[sk, sq]    = kT.T @ qT             (K=64)
  eT            = exp(sT/8) * causal_mask
  avT[hd+1, sq] = v_aug.T @ eT          (v augmented with a ones row ->
                                         numerator and denominator in one
                                         accumulation)
  yT            = avT[0:64] * bcast(1/avT[64])
  out[sq, d]    = yT.T @ woT            (partial; host sums over cores)
"""

import sys

sys.path.insert(0, "/opt/trn_rl_repo")

import numpy as np
import ml_dtypes

import concourse.bass as bass
import concourse.bacc as bacc
import concourse.mybir as mybir
import concourse.tile as tile

BF16 = ml_dtypes.bfloat16

S = 2048
D = 2048
HD = 64
NH = 32
NKV = 8
NCORES = 8
QH = NH // NCORES            # 4 local q heads
QCOLS = QH * HD              # 256
KVCOLS = 2 * HD              # 128 (k and v, one kv head)
P = 128                      # partitions
NK = D // P                  # 16 contraction tiles
NSQ = S // P                 # 16 seq tiles of 128
NCH = 4                      # seq chunks of 512
CH = 512

_CACHE = {}


def _build():
    mm_dt = mybir.dt.bfloat16
    f32 = mybir.dt.float32

    nc = bacc.Bacc()
    xt_d = nc.dram_tensor("xt", [D, S], mm_dt, kind="ExternalInput")
    wqkvt_d = nc.dram_tensor("wqkvt", [D, QCOLS + KVCOLS], mm_dt, kind="ExternalInput")
    wot_d = nc.dram_tensor("wot", [QCOLS, D], mm_dt, kind="ExternalInput")
    cos_d = nc.dram_tensor("cos128", [P, S], f32, kind="ExternalInput")
    sinsw_d = nc.dram_tensor("sinsw128", [P, S], f32, kind="ExternalInput")
    masks_d = nc.dram_tensor("masks", [P, 4 * CH], mm_dt, kind="ExternalInput")
    ones_d = nc.dram_tensor("ones64", [1, HD], f32, kind="ExternalInput")
    ident_d = nc.dram_tensor("ident", [HD, HD], mm_dt, kind="ExternalInput")
    out_d = nc.dram_tensor("out", [S, D], f32, kind="ExternalOutput")

    with tile.TileContext(nc) as tc:
        with (
            tc.tile_pool(name="const", bufs=1) as cpool,
            tc.tile_pool(name="xt", bufs=NK) as xpool,
            tc.tile_pool(name="wq", bufs=NK) as wpool,
            tc.tile_pool(name="big", bufs=1) as bigpool,
            tc.tile_pool(name="vaug", bufs=NSQ) as vpool,
            tc.tile_pool(name="et", bufs=20) as epool,
            tc.tile_pool(name="tmp", bufs=3) as tpool,
            tc.tile_pool(name="ps_a", bufs=3, space="PSUM") as ps_a,
            tc.tile_pool(name="ps_s", bufs=3, space="PSUM") as ps_s,
            tc.tile_pool(name="ps_av", bufs=2, space="PSUM") as ps_av,
        ):
            # ---- constants / weights in ----
            cos_sb = cpool.tile([P, S], f32, tag="cos")
            sinsw_sb = cpool.tile([P, S], f32, tag="sinsw")
            masks_sb = cpool.tile([P, 4 * CH], mm_dt, tag="masks")
            ones_sb = cpool.tile([1, HD], f32, tag="ones")
            ident_sb = cpool.tile([HD, HD], mm_dt, tag="ident")
            nc.sync.dma_start(cos_sb[:], cos_d[:])
            nc.sync.dma_start(sinsw_sb[:], sinsw_d[:])
            nc.sync.dma_start(masks_sb[:], masks_d[:])
            nc.sync.dma_start(ones_sb[:], ones_d[:])
            nc.sync.dma_start(ident_sb[:], ident_d[:])

            wot_sb = []
            for k in range(2):
                t = cpool.tile([P, D], mm_dt, tag=f"wot{k}", name=f"wot{k}")
                nc.sync.dma_start(t[:], wot_d[k * P : (k + 1) * P, :])
                wot_sb.append(t)

            xt_sb = []
            wq_sb = []
            for k in range(NK):
                t = xpool.tile([P, S], mm_dt, tag="xt", name=f"xt{k}")
                nc.sync.dma_start(t[:], xt_d[k * P : (k + 1) * P, :])
                xt_sb.append(t)
                w = wpool.tile([P, QCOLS + KVCOLS], mm_dt, tag="wq", name=f"wqkv{k}")
                nc.sync.dma_start(w[:], wqkvt_d[k * P : (k + 1) * P, :])
                wq_sb.append(w)

            qt_sb = [bigpool.tile([HD, S], mm_dt, tag=f"qt{m}", name=f"qt{m}") for m in range(QH)]
            kt_sb = bigpool.tile([HD, S], mm_dt, tag="kt")
            vt_sb = bigpool.tile([HD, S], mm_dt, tag="vt")
            yt_sb = [bigpool.tile([P, S], mm_dt, tag=f"yt{m}", name=f"yt{m}") for m in range(2)]

            # ---- projections: qkvT[col, sq] accumulated over d ----
            # Mtile 0: q heads 0,1 | Mtile 1: q heads 2,3 | Mtile 2: [kT; vT]
            def rope(dst, ps, chunk):
                # dst[:, chunk] = RoPE(ps) for one 64-row de-interleaved head
                t2 = tpool.tile([HD, CH], f32, tag="rope_t2", name="rope_t2")
                nc.vector.tensor_mul(t2[0:32, :], ps[32:64, :], sinsw_sb[0:32, chunk])
                nc.vector.tensor_mul(t2[32:64, :], ps[0:32, :], sinsw_sb[32:64, chunk])
                nc.vector.tensor_mul(dst[:, chunk], ps[:], cos_sb[0:HD, chunk])
                nc.vector.tensor_add(dst[:, chunk], dst[:, chunk], t2[:])

            for m in range(3):
                for j in range(NCH):
                    chunk = slice(j * CH, (j + 1) * CH)
                    ps = ps_a.tile([P, CH], f32, tag="proj", name="ps_proj")
                    with nc.named_scope("proj"):
                     for k in range(NK):
                        nc.tensor.matmul(
                            ps[:],
                            wq_sb[k][:, m * P : (m + 1) * P],
                            xt_sb[k][:, chunk],
                            start=(k == 0),
                            stop=(k == NK - 1),
                        )
                    with nc.named_scope("rope"):
                     if m < 2:
                        rope(qt_sb[2 * m], ps[0:HD, :], chunk)
                        rope(qt_sb[2 * m + 1], ps[HD:P, :], chunk)
                     else:
                        rope(kt_sb, ps[0:HD, :], chunk)
                        nc.any.tensor_copy(vt_sb[:, chunk], ps[HD:P, :])

            # ---- v transpose to natural layout + ones row ----
            vaug_sb = []
            for i in range(NSQ):
              with nc.named_scope("vtrans"):
                pt = ps_av.tile([P, HD], mm_dt, tag="av", name="ps_vt")
                nc.tensor.transpose(pt[:], vt_sb[:, i * P : (i + 1) * P], ident_sb[:])
                va = vpool.tile([P, HD + 1], mm_dt, tag="vaug", name=f"vaug{i}")
                nc.any.tensor_copy(va[:, 0:HD], pt[:])
                nc.gpsimd.memset(va[:, HD : HD + 1], 1.0)
                vaug_sb.append(va)

            # ---- SDPA per (head, sq-chunk), causal ----
            for h in range(QH):
                qrow = (h % 2) * HD
                for j in range(NCH):
                    chunk = slice(j * CH, (j + 1) * CH)
                    nlive = 4 * j + 4  # sk tiles 0..4j+3 are causal-live
                    ets = []
                    for i in range(nlive):
                        ps = ps_s.tile([P, CH], f32, tag="sc", name="ps_sc")
                        with nc.named_scope("scores"):
                         nc.tensor.matmul(
                            ps[:],
                            kt_sb[:, i * P : (i + 1) * P],
                            qt_sb[h][:, chunk],
                            start=True,
                            stop=True,
                         )
                        et = epool.tile([P, CH], mm_dt, tag="et", name="et")
                        with nc.named_scope("exp"):
                         nc.scalar.activation(
                            et[:], ps[:], mybir.ActivationFunctionType.Exp, scale=0.125
                        )
                        o = i - 4 * j
                        if o >= 0:
                            with nc.named_scope("mask"):
                             nc.vector.tensor_mul(
                                et[:], et[:], masks_sb[:, o * CH : (o + 1) * CH]
                             )
                        ets.append(et)
                    pav = ps_av.tile([HD + 1, CH], f32, tag="av", name="ps_av")
                    with nc.named_scope("av"):
                     for i in range(nlive):
                        nc.tensor.matmul(
                            pav[:],
                            vaug_sb[i][:],
                            ets[i][:],
                            start=(i == 0),
                            stop=(i == nlive - 1),
                        )
                    # normalize: yT = avT[0:64] / avT[64]
                    with nc.named_scope("norm"):
                        recip = tpool.tile([1, CH], f32, tag="recip", name="recip")
                        nc.vector.reciprocal(recip[:], pav[HD : HD + 1, :])
                        pb = ps_av.tile([HD, CH], f32, tag="av", name="ps_bc")
                        nc.tensor.matmul(pb[:], ones_sb[:], recip[:], start=True, stop=True)
                        bc = tpool.tile([HD, CH], f32, tag="bc", name="bc")
                        nc.any.tensor_copy(bc[:], pb[:])
                        nc.vector.tensor_mul(
                            yt_sb[h // 2][qrow : qrow + HD, chunk], pav[0:HD, :], bc[:]
                        )

            # ---- output projection partial: out[sq, d] ----
            for sm in range(NSQ):
                srow = slice(sm * P, (sm + 1) * P)
                for dcJ in range(NCH):
                    dch = slice(dcJ * CH, (dcJ + 1) * CH)
                    pw = ps_a.tile([P, CH], f32, tag="proj", name="ps_wo")
                    with nc.named_scope("wo"):
                     for k in range(2):
                        nc.tensor.matmul(
                            pw[:],
                            yt_sb[k][:, srow],
                            wot_sb[k][:, dch],
                            start=(k == 0),
                            stop=(k == 1),
                        )
                    ot = tpool.tile([P, CH], f32, tag="ot", name="ot")
                    with nc.named_scope("outdma"):
                        nc.any.tensor_copy(ot[:], pw[:])
                        nc.sync.dma_start(out_d[srow, dch], ot[:])

    nc.finalize()
    return nc


def _host_inputs(x, freqs_cos, freqs_sin, wq, wk, wv, wo):
    """Build the 8 per-core input maps (all host-side preprocessing)."""
    x = np.asarray(x, np.float32)
    cos = np.asarray(freqs_cos, np.float32)  # [S, 32]
    sin = np.asarray(freqs_sin, np.float32)
    wq = np.asarray(wq, np.float32)
    wk = np.asarray(wk, np.float32)
    wv = np.asarray(wv, np.float32)
    wo = np.asarray(wo, np.float32)

    perm = np.concatenate([np.arange(0, HD, 2), np.arange(1, HD, 2)])  # de-interleave

    xt = np.ascontiguousarray(x[0].T).astype(BF16)

    # cos128[d, t] = cos[t, d % 32]; sinsw has -sin for the real half rows
    cos128 = np.empty((P, S), np.float32)
    sinsw = np.empty((P, S), np.float32)
    for dd in range(P):
        i = dd % 32
        cos128[dd] = cos[:, i]
        sinsw[dd] = (-sin[:, i]) if (dd % HD) < 32 else sin[:, i]

    masks = np.zeros((P, 4 * CH), np.float32)
    pp = np.arange(P)[:, None]
    ff = np.arange(CH)[None, :]
    for o in range(4):
        masks[:, o * CH : (o + 1) * CH] = (128 * o + pp <= ff).astype(np.float32)
    masks = masks.astype(BF16)

    ones64 = np.ones((1, HD), np.float32)
    ident = np.eye(HD, dtype=np.float32).astype(BF16)

    in_maps = []
    for c in range(NCORES):
        wq_c = wq[c * QCOLS : (c + 1) * QCOLS].reshape(QH, HD, D)[:, perm, :].reshape(
            QCOLS, D
        )
        wk_c = wk[c * HD : (c + 1) * HD][perm, :]
        wv_c = wv[c * HD : (c + 1) * HD]
        wqkvt = np.ascontiguousarray(
            np.concatenate([wq_c, wk_c, wv_c], axis=0).T
        ).astype(BF16)
        wot = np.ascontiguousarray(wo[:, c * QCOLS : (c + 1) * QCOLS].T).astype(BF16)
        in_maps.append(
            {
                "xt": xt,
                "wqkvt": wqkvt,
                "wot": wot,
                "cos128": cos128,
                "sinsw128": sinsw,
                "masks": masks,
                "ones64": ones64,
                "ident": ident,
            }
        )
    return in_maps


def kernel(x, freqs_cos, freqs_sin, wq, wk, wv, wo):
    from concourse.bass_utils import run_bass_kernel_spmd

    if "nc" not in _CACHE:
        _CACHE["nc"] = _build()
    nc = _CACHE["nc"]
    in_maps = _host_inputs(x, freqs_cos, freqs_sin, wq, wk, wv, wo)
    res = run_bass_kernel_spmd(nc, in_maps, core_ids=list(range(NCORES)))
    out = np.zeros((S, D), np.float64)
    for r in res.results:
        out += r["out"].astype(np.float64)
    return out.astype(np.float32).reshape(1, S, D)


# revision 21
# speedup vs baseline: 8.8280x; 8.8280x over previous
"""GQA causal attention (llama3-style RoPE) on 8 TRN2 NeuronCores.

Sharding: tensor-parallel over heads. Core c gets q-heads 4c..4c+3 and
kv-head c (GQA groups intact), plus the matching row-block of wo.T.
Each core computes a full [S, D] partial of the output projection;
the host sums the 8 partials (the "all-reduce" of the row-sharded wo).

Per-core pipeline (all layouts chosen so no on-device transpose of x/q/k
is ever needed):
  qkvT[col, s]  = wqkvT.T @ xT          (weights stationary, xT streaming)
  RoPE on qT/kT (de-interleaved pair layout via host-permuted weight rows)
  sT[sk, sq]    = kT.T @ qT             (K=64)
  eT            = exp(sT/8) * causal_mask
  avT[hd+1, sq] = v_aug.T @ eT          (v augmented with a ones row ->
                                         numerator and denominator in one
                                         accumulation)
  yT            = avT[0:64] * bcast(1/avT[64])
  out[sq, d]    = yT.T @ woT            (partial; host sums over cores)
"""

import sys

for _p in ("/opt/trn_rl_repo", "/root/.axon_site/_ro/trn_rl_repo"):
    if _p not in sys.path:
        sys.path.insert(0, _p)

import numpy as np
import ml_dtypes

import concourse.bass as bass
import concourse.bacc as bacc
import concourse.mybir as mybir
import concourse.tile as tile

BF16 = ml_dtypes.bfloat16

S = 2048
D = 2048
HD = 64
NH = 32
NKV = 8
NCORES = 8
QH = NH // NCORES            # 4 local q heads
QCOLS = QH * HD              # 256
KVCOLS = 2 * HD              # 128 (k and v, one kv head)
P = 128                      # partitions
NK = D // P                  # 16 contraction tiles
NSQ = S // P                 # 16 seq tiles of 128
NCH = 4                      # seq chunks of 512
CH = 512

_CACHE = {}


def _build():
    mm_dt = mybir.dt.bfloat16
    f16 = mybir.dt.float16
    f32 = mybir.dt.float32

    nc = bacc.Bacc()
    xt_d = nc.dram_tensor("xt", [D, S], mm_dt, kind="ExternalInput")
    wqkvt_d = nc.dram_tensor("wqkvt", [D, QCOLS + KVCOLS], mm_dt, kind="ExternalInput")
    wot_d = nc.dram_tensor("wot", [QCOLS, D], mm_dt, kind="ExternalInput")
    cos_d = nc.dram_tensor("cos64", [HD, S], f16, kind="ExternalInput")
    swap_d = nc.dram_tensor("swap64", [HD, S], f16, kind="ExternalInput")
    masks_d = nc.dram_tensor("masks", [P, P], mm_dt, kind="ExternalInput")
    ones_d = nc.dram_tensor("ones64", [1, HD], f32, kind="ExternalInput")
    ident_d = nc.dram_tensor("ident", [HD, HD], mm_dt, kind="ExternalInput")
    out_d = nc.dram_tensor("out", [S, D], f32, kind="ExternalOutput")

    with tile.TileContext(nc) as tc:
        with (
            tc.tile_pool(name="const", bufs=1) as cpool,
            tc.tile_pool(name="xt", bufs=NK) as xpool,
            tc.tile_pool(name="wq", bufs=NK) as wpool,
            tc.tile_pool(name="big", bufs=1) as bigpool,
            tc.tile_pool(name="vaug", bufs=NSQ) as vpool,
            tc.tile_pool(name="et", bufs=20) as epool,
            tc.tile_pool(name="tmp", bufs=3) as tpool,
            tc.tile_pool(name="ps_a", bufs=2, space="PSUM") as ps_a,
            tc.tile_pool(name="ps_s", bufs=2, space="PSUM") as ps_s,
            tc.tile_pool(name="ps_av", bufs=2, space="PSUM") as ps_av,
        ):
            # ---- constants / weights in ----
            # small tables via SWDGE (gpsimd); bulk via the two HWDGE
            # queues (SP + ACT) in parallel
            cos_sb = cpool.tile([HD, S], f16, tag="cos")
            swap_sb = cpool.tile([HD, S], f16, tag="swap")
            masks_sb = cpool.tile([P, P], mm_dt, tag="masks")
            ones_sb = cpool.tile([1, HD], f32, tag="ones")
            ident_sb = cpool.tile([HD, HD], mm_dt, tag="ident")
            zbias = cpool.tile([P, 1], f32, tag="zbias")
            nc.gpsimd.memset(zbias[:], 0.0)
            nc.gpsimd.dma_start(cos_sb[:], cos_d[:])
            nc.gpsimd.dma_start(swap_sb[:], swap_d[:])
            nc.gpsimd.dma_start(masks_sb[:], masks_d[:])
            nc.gpsimd.dma_start(ones_sb[:], ones_d[:])
            nc.gpsimd.dma_start(ident_sb[:], ident_d[:])

            wot_sb = []
            for k in range(2):
                t = cpool.tile([P, D], mm_dt, tag=f"wot{k}", name=f"wot{k}")
                nc.gpsimd.dma_start(t[:], wot_d[k * P : (k + 1) * P, :])
                wot_sb.append(t)

            hwdge = [nc.sync, nc.scalar]
            xt_sb = []
            wq_sb = []
            for k in range(NK):
                w = wpool.tile([P, QCOLS + KVCOLS], mm_dt, tag="wq", name=f"wqkv{k}")
                hwdge[k % 2].dma_start(w[:], wqkvt_d[k * P : (k + 1) * P, :])
                wq_sb.append(w)
            for k in range(NK):
                xt_sb.append(xpool.tile([P, S], mm_dt, tag="xt", name=f"xt{k}"))
            for q in range(NCH):
                qs = slice(q * CH, (q + 1) * CH)
                for k in range(NK):
                    hwdge[k % 2].dma_start(xt_sb[k][:, qs], xt_d[k * P : (k + 1) * P, qs])

            qt_sb = [bigpool.tile([P, S], f16, tag=f"qt{m}", name=f"qt{m}") for m in range(QH)]
            kt_sb = bigpool.tile([P, S], f16, tag="kt")
            vt_sb = bigpool.tile([HD, S], mm_dt, tag="vt")
            yt_sb = [bigpool.tile([P, S], mm_dt, tag=f"yt{m}", name=f"yt{m}") for m in range(2)]

            # ---- projections: qkvT[col, sq] accumulated over d ----
            # Mtile order: kv first so SDPA can start as soon as q is ready.
            # Mtile 2: [kT; vT] | Mtile 0: q heads 0,1 | Mtile 1: q heads 2,3
            def rope(dst, ps, chunk):
                # dst[:, chunk] = RoPE(ps) for one 64-row de-interleaved head.
                # Drain psum to f16 SBUF once so the elementwise ops run in
                # the DVE 2-byte SBUF fast mode.
                qr = tpool.tile([HD, CH], f16, tag="rope_qr", name="rope_qr")
                nc.vector.tensor_copy(qr[:], ps[:])
                t2 = tpool.tile([HD, CH], f16, tag="rope_t2", name="rope_t2")
                nc.vector.tensor_mul(t2[0:32, :], qr[32:64, :], swap_sb[32:64, chunk])
                nc.vector.tensor_mul(t2[32:64, :], qr[0:32, :], swap_sb[0:32, chunk])
                nc.vector.tensor_mul(dst[:, chunk], qr[:], cos_sb[:, chunk])
                nc.vector.tensor_add(dst[:, chunk], dst[:, chunk], t2[:])

            vaug_sb = [None] * NSQ

            def vtrans(jlist):
                with nc.named_scope("vtrans"):
                    for i in jlist:
                        pt = ps_av.tile([P, HD], mm_dt, tag="av", name="ps_vt")
                        nc.tensor.transpose(
                            pt[:], vt_sb[:, i * P : (i + 1) * P], ident_sb[:]
                        )
                        va = vpool.tile([P, HD + 1], mm_dt, tag="vaug", name=f"vaug{i}")
                        nc.vector.tensor_copy(va[:, 0:HD], pt[:])
                        nc.gpsimd.memset(va[:, HD : HD + 1], 1.0)
                        vaug_sb[i] = va

            for m in (2, 0, 1):
                if m == 1:
                    vtrans(range(NSQ))
                for j in range(NCH):
                    chunk = slice(j * CH, (j + 1) * CH)
                    ps = ps_a.tile([P, CH], f32, tag="proj", name="ps_proj")
                    with nc.named_scope("proj"):
                        for k in range(NK):
                            nc.tensor.matmul(
                                ps[:],
                                wq_sb[k][:, m * P : (m + 1) * P],
                                xt_sb[k][:, chunk],
                                start=(k == 0),
                                stop=(k == NK - 1),
                            )
                    with nc.named_scope("rope"):
                        if m < 2:
                            rope(qt_sb[2 * m][0:HD, :], ps[0:HD, :], chunk)
                            rope(qt_sb[2 * m + 1][0:HD, :], ps[HD:P, :], chunk)
                            for hh in (2 * m, 2 * m + 1):
                                nc.vector.tensor_copy(
                                    qt_sb[hh][HD:P, chunk], qt_sb[hh][0:HD, chunk]
                                )
                        else:
                            rope(kt_sb[0:HD, :], ps[0:HD, :], chunk)
                            nc.vector.tensor_copy(kt_sb[HD:P, chunk], kt_sb[0:HD, chunk])
                            nc.vector.tensor_copy(vt_sb[:, chunk], ps[HD:P, :])

            # ---- SDPA per (head, sq-chunk), causal ----
            # sk-tile pairs run concurrently in the PE array via row groups
            # (K=64): pair element 0 in rows 0-63, element 1 in rows 64-127.
            # Each pair writes one [128, 1024] 2-bank psum tile so the exp
            # over both halves is a single ACT op. Boundary tiles
            # (o = i-4j >= 0) only compute/exp columns [128*o:512); the
            # first 128 of those get the triangular mask.
            for j in range(NCH):
                for h in range(QH):
                    qrow = (h % 2) * HD
                    chunk = slice(j * CH, (j + 1) * CH)
                    nlive = 4 * j + 4  # sk tiles 0..4j+3 are causal-live
                    offs = [max(0, (i - 4 * j)) * P for i in range(nlive)]
                    ets = []
                    with nc.named_scope("scores"):
                        for i in range(0, nlive, 2):
                            ps2 = ps_s.tile([P, 2 * CH], f32, tag="sc", name="ps_sc")
                            for u in range(2):
                                off = offs[i + u]
                                rg = slice(u * HD, (u + 1) * HD)
                                nc.tensor.matmul(
                                    ps2[:, u * CH + off : (u + 1) * CH],
                                    kt_sb[rg, (i + u) * P : (i + u + 1) * P],
                                    qt_sb[h][rg, j * CH + off : (j + 1) * CH],
                                    start=True,
                                    stop=True,
                                )
                            et2 = epool.tile([P, 2 * CH], mm_dt, tag="et", name="et")
                            with nc.named_scope("exp"):
                                if offs[i] == 0 and offs[i + 1] == 0:
                                    nc.scalar.activation(
                                        et2[:],
                                        ps2[:],
                                        mybir.ActivationFunctionType.Exp,
                                        bias=zbias[:],
                                        scale=0.125,
                                    )
                                else:
                                    for u in range(2):
                                        off = offs[i + u]
                                        nc.scalar.activation(
                                            et2[:, u * CH + off : (u + 1) * CH],
                                            ps2[:, u * CH + off : (u + 1) * CH],
                                            mybir.ActivationFunctionType.Exp,
                                            bias=zbias[:],
                                            scale=0.125,
                                        )
                            for u in range(2):
                                if i + u >= nlive - 4:  # boundary tile
                                    off = u * CH + offs[i + u]
                                    with nc.named_scope("mask"):
                                        nc.vector.tensor_mul(
                                            et2[:, off : off + P],
                                            et2[:, off : off + P],
                                            masks_sb[:],
                                        )
                            ets.append(et2)
                    pav = ps_av.tile([HD + 1, CH], f32, tag="av", name="ps_av")
                    with nc.named_scope("av"):
                        for i in range(nlive):
                            off = offs[i]
                            nc.tensor.matmul(
                                pav[:, off:],
                                vaug_sb[i][:],
                                ets[i // 2][:, (i % 2) * CH + off : (i % 2 + 1) * CH],
                                start=(i == 0),
                                stop=(i == nlive - 1),
                            )
                    # normalize: yT = avT[0:64] / avT[64]
                    with nc.named_scope("norm"):
                        recip = tpool.tile([1, CH], f32, tag="recip", name="recip")
                        nc.vector.reciprocal(recip[:], pav[HD : HD + 1, :])
                        bc = tpool.tile([HD, CH], f32, tag="bc", name="bc")
                        nc.gpsimd.partition_broadcast(bc[:], recip[:])
                        nc.vector.tensor_mul(
                            yt_sb[h // 2][qrow : qrow + HD, chunk], pav[0:HD, :], bc[:]
                        )

            # ---- output projection partial: out[sq, d] ----
            for sm in range(NSQ):
                srow = slice(sm * P, (sm + 1) * P)
                for dcJ in range(NCH):
                    dch = slice(dcJ * CH, (dcJ + 1) * CH)
                    pw = ps_a.tile([P, CH], f32, tag="proj", name="ps_wo")
                    with nc.named_scope("wo"):
                        for k in range(2):
                            nc.tensor.matmul(
                                pw[:],
                                yt_sb[k][:, srow],
                                wot_sb[k][:, dch],
                                start=(k == 0),
                                stop=(k == 1),
                            )
                    ot = tpool.tile([P, CH], f32, tag="ot", name="ot")
                    with nc.named_scope("outdma"):
                        nc.vector.tensor_copy(ot[:], pw[:])
                        odma = [nc.sync, nc.gpsimd][(sm * NCH + dcJ) % 2]
                        odma.dma_start(out_d[srow, dch], ot[:])

    nc.finalize()
    return nc


def _host_inputs(x, freqs_cos, freqs_sin, wq, wk, wv, wo):
    """Build the 8 per-core input maps (all host-side preprocessing)."""
    x = np.asarray(x, np.float32)
    cos = np.asarray(freqs_cos, np.float32)  # [S, 32]
    sin = np.asarray(freqs_sin, np.float32)
    wq = np.asarray(wq, np.float32)
    wk = np.asarray(wk, np.float32)
    wv = np.asarray(wv, np.float32)
    wo = np.asarray(wo, np.float32)

    perm = np.concatenate([np.arange(0, HD, 2), np.arange(1, HD, 2)])  # de-interleave

    xt = np.ascontiguousarray(x[0].T).astype(BF16)

    # cos64[d, t] = cos[t, d % 32]; swap64 rows 0:32 = +sin (imag-out),
    # rows 32:64 = -sin (real-out) so both tensor_mul inputs share a base
    # partition (walrus SB-SB constraint)
    cos64 = np.empty((HD, S), np.float16)
    swap64 = np.empty((HD, S), np.float16)
    for dd in range(HD):
        i = dd % 32
        cos64[dd] = cos[:, i]
        swap64[dd] = sin[:, i] if dd < 32 else -sin[:, i]

    pp = np.arange(P)[:, None]
    ff = np.arange(P)[None, :]
    masks = (pp <= ff).astype(np.float32).astype(BF16)

    ones64 = np.ones((1, HD), np.float32)
    ident = np.eye(HD, dtype=np.float32).astype(BF16)

    in_maps = []
    for c in range(NCORES):
        wq_c = wq[c * QCOLS : (c + 1) * QCOLS].reshape(QH, HD, D)[:, perm, :].reshape(
            QCOLS, D
        )
        wk_c = wk[c * HD : (c + 1) * HD][perm, :]
        wv_c = wv[c * HD : (c + 1) * HD]
        wqkvt = np.ascontiguousarray(
            np.concatenate([wq_c, wk_c, wv_c], axis=0).T
        ).astype(BF16)
        wot = np.ascontiguousarray(wo[:, c * QCOLS : (c + 1) * QCOLS].T).astype(BF16)
        in_maps.append(
            {
                "xt": xt,
                "wqkvt": wqkvt,
                "wot": wot,
                "cos64": cos64,
                "swap64": swap64,
                "masks": masks,
                "ones64": ones64,
                "ident": ident,
            }
        )
    return in_maps


def kernel(x, freqs_cos, freqs_sin, wq, wk, wv, wo):
    from concourse.bass_utils import run_bass_kernel_spmd

    if "nc" not in _CACHE:
        _CACHE["nc"] = _build()
    nc = _CACHE["nc"]
    in_maps = _host_inputs(x, freqs_cos, freqs_sin, wq, wk, wv, wo)
    res = run_bass_kernel_spmd(nc, in_maps, core_ids=list(range(NCORES)))
    out = np.zeros((S, D), np.float64)
    for r in res.results:
        out += r["out"].astype(np.float64)
    return out.astype(np.float32).reshape(1, S, D)


# revision 22
# speedup vs baseline: 8.9577x; 1.0147x over previous
"""GQA causal attention (llama3-style RoPE) on 8 TRN2 NeuronCores.

Sharding: tensor-parallel over heads. Core c gets q-heads 4c..4c+3 and
kv-head c (GQA groups intact), plus the matching row-block of wo.T.
Each core computes a full [S, D] partial of the output projection;
the host sums the 8 partials (the "all-reduce" of the row-sharded wo).

Per-core pipeline (all layouts chosen so no on-device transpose of x/q/k
is ever needed):
  qkvT[col, s]  = wqkvT.T @ xT          (weights stationary, xT streaming)
  RoPE on qT/kT (de-interleaved pair layout via host-permuted weight rows)
  sT[sk, sq]    = kT.T @ qT             (K=64)
  eT            = exp(sT/8) * causal_mask
  avT[hd+1, sq] = v_aug.T @ eT          (v augmented with a ones row ->
                                         numerator and denominator in one
                                         accumulation)
  yT            = avT[0:64] * bcast(1/avT[64])
  out[sq, d]    = yT.T @ woT            (partial; host sums over cores)
"""

import sys

for _p in ("/opt/trn_rl_repo", "/root/.axon_site/_ro/trn_rl_repo"):
    if _p not in sys.path:
        sys.path.insert(0, _p)

import numpy as np
import ml_dtypes

import concourse.bass as bass
import concourse.bacc as bacc
import concourse.mybir as mybir
import concourse.tile as tile

BF16 = ml_dtypes.bfloat16

S = 2048
D = 2048
HD = 64
NH = 32
NKV = 8
NCORES = 8
QH = NH // NCORES            # 4 local q heads
QCOLS = QH * HD              # 256
KVCOLS = 2 * HD              # 128 (k and v, one kv head)
P = 128                      # partitions
NK = D // P                  # 16 contraction tiles
NSQ = S // P                 # 16 seq tiles of 128
NCH = 4                      # seq chunks of 512
CH = 512

_CACHE = {}


def _build():
    mm_dt = mybir.dt.bfloat16
    f16 = mybir.dt.float16
    f32 = mybir.dt.float32

    nc = bacc.Bacc()
    xt_d = nc.dram_tensor("xt", [D, S], mm_dt, kind="ExternalInput")
    wqkvt_d = nc.dram_tensor("wqkvt", [D, QCOLS + KVCOLS], mm_dt, kind="ExternalInput")
    wot_d = nc.dram_tensor("wot", [QCOLS, D], mm_dt, kind="ExternalInput")
    cos_d = nc.dram_tensor("cos64", [HD, S], f16, kind="ExternalInput")
    swap_d = nc.dram_tensor("swap64", [HD, S], f16, kind="ExternalInput")
    masks_d = nc.dram_tensor("masks", [P, P], mm_dt, kind="ExternalInput")
    ones_d = nc.dram_tensor("ones64", [1, HD], f32, kind="ExternalInput")
    ident_d = nc.dram_tensor("ident", [HD, HD], mm_dt, kind="ExternalInput")
    out_d = nc.dram_tensor("out", [S, D], f32, kind="ExternalOutput")

    with tile.TileContext(nc) as tc:
        with (
            tc.tile_pool(name="const", bufs=1) as cpool,
            tc.tile_pool(name="xt", bufs=NK) as xpool,
            tc.tile_pool(name="wq", bufs=NK) as wpool,
            tc.tile_pool(name="big", bufs=1) as bigpool,
            tc.tile_pool(name="vaug", bufs=NSQ) as vpool,
            tc.tile_pool(name="et", bufs=20) as epool,
            tc.tile_pool(name="tmp", bufs=3) as tpool,
            tc.tile_pool(name="ps_a", bufs=2, space="PSUM") as ps_a,
            tc.tile_pool(name="ps_s", bufs=2, space="PSUM") as ps_s,
            tc.tile_pool(name="ps_av", bufs=2, space="PSUM") as ps_av,
        ):
            # ---- constants / weights in ----
            # small tables via SWDGE (gpsimd); bulk via the two HWDGE
            # queues (SP + ACT) in parallel
            cos_sb = cpool.tile([HD, S], f16, tag="cos")
            swap_sb = cpool.tile([HD, S], f16, tag="swap")
            masks_sb = cpool.tile([P, P], mm_dt, tag="masks")
            ones_sb = cpool.tile([1, HD], f32, tag="ones")
            ident_sb = cpool.tile([HD, HD], mm_dt, tag="ident")
            zbias = cpool.tile([P, 1], f32, tag="zbias")
            nc.gpsimd.memset(zbias[:], 0.0)
            nc.gpsimd.dma_start(cos_sb[:], cos_d[:])
            nc.gpsimd.dma_start(swap_sb[:], swap_d[:])
            nc.gpsimd.dma_start(masks_sb[:], masks_d[:])
            nc.gpsimd.dma_start(ones_sb[:], ones_d[:])
            nc.gpsimd.dma_start(ident_sb[:], ident_d[:])

            hwdge = [nc.sync, nc.scalar]
            xt_sb = []
            wq_sb = []
            for k in range(NK):
                w = wpool.tile([P, QCOLS + KVCOLS], mm_dt, tag="wq", name=f"wqkv{k}")
                hwdge[k % 2].dma_start(w[:], wqkvt_d[k * P : (k + 1) * P, :])
                wq_sb.append(w)
            for k in range(NK):
                xt_sb.append(xpool.tile([P, S], mm_dt, tag="xt", name=f"xt{k}"))
            for q in range(NCH):
                qs = slice(q * CH, (q + 1) * CH)
                for k in range(NK):
                    hwdge[k % 2].dma_start(xt_sb[k][:, qs], xt_d[k * P : (k + 1) * P, qs])

            wot_sb = []
            for k in range(2):
                t = cpool.tile([P, D], mm_dt, tag=f"wot{k}", name=f"wot{k}")
                nc.gpsimd.dma_start(t[:], wot_d[k * P : (k + 1) * P, :])
                wot_sb.append(t)

            qt_sb = [bigpool.tile([P, S], f16, tag=f"qt{m}", name=f"qt{m}") for m in range(QH)]
            kt_sb = bigpool.tile([P, S], f16, tag="kt")
            vt_sb = bigpool.tile([HD, S], mm_dt, tag="vt")
            yt_sb = [bigpool.tile([P, S], mm_dt, tag=f"yt{m}", name=f"yt{m}") for m in range(2)]

            # ---- projections: qkvT[col, sq] accumulated over d ----
            # Mtile order: kv first so SDPA can start as soon as q is ready.
            # Mtile 2: [kT; vT] | Mtile 0: q heads 0,1 | Mtile 1: q heads 2,3
            def rope(dst, ps, chunk):
                # dst[:, chunk] = RoPE(ps) for one 64-row de-interleaved head.
                # Drain psum to f16 SBUF once so the elementwise ops run in
                # the DVE 2-byte SBUF fast mode.
                qr = tpool.tile([HD, CH], f16, tag="rope_qr", name="rope_qr")
                nc.vector.tensor_copy(qr[:], ps[:])
                t2 = tpool.tile([HD, CH], f16, tag="rope_t2", name="rope_t2")
                nc.vector.tensor_mul(t2[0:32, :], qr[32:64, :], swap_sb[32:64, chunk])
                nc.vector.tensor_mul(t2[32:64, :], qr[0:32, :], swap_sb[0:32, chunk])
                nc.vector.tensor_mul(dst[:, chunk], qr[:], cos_sb[:, chunk])
                nc.vector.tensor_add(dst[:, chunk], dst[:, chunk], t2[:])

            vaug_sb = [None] * NSQ

            def vtrans(jlist):
                with nc.named_scope("vtrans"):
                    for i in jlist:
                        pt = ps_av.tile([P, HD], mm_dt, tag="av", name="ps_vt")
                        nc.tensor.transpose(
                            pt[:], vt_sb[:, i * P : (i + 1) * P], ident_sb[:]
                        )
                        va = vpool.tile([P, HD + 1], mm_dt, tag="vaug", name=f"vaug{i}")
                        nc.vector.tensor_copy(va[:, 0:HD], pt[:])
                        nc.gpsimd.memset(va[:, HD : HD + 1], 1.0)
                        vaug_sb[i] = va

            for m in (2, 0, 1):
                if m == 1:
                    vtrans(range(NSQ))
                for j in range(NCH):
                    chunk = slice(j * CH, (j + 1) * CH)
                    ps = ps_a.tile([P, CH], f32, tag="proj", name="ps_proj")
                    with nc.named_scope("proj"):
                        for k in range(NK):
                            nc.tensor.matmul(
                                ps[:],
                                wq_sb[k][:, m * P : (m + 1) * P],
                                xt_sb[k][:, chunk],
                                start=(k == 0),
                                stop=(k == NK - 1),
                            )
                    with nc.named_scope("rope"):
                        if m < 2:
                            rope(qt_sb[2 * m][0:HD, :], ps[0:HD, :], chunk)
                            rope(qt_sb[2 * m + 1][0:HD, :], ps[HD:P, :], chunk)
                            for hh in (2 * m, 2 * m + 1):
                                nc.gpsimd.dma_start(
                                    qt_sb[hh][HD:P, chunk], qt_sb[hh][0:HD, chunk]
                                )
                        else:
                            rope(kt_sb[0:HD, :], ps[0:HD, :], chunk)
                            nc.gpsimd.dma_start(kt_sb[HD:P, chunk], kt_sb[0:HD, chunk])
                            nc.vector.tensor_copy(vt_sb[:, chunk], ps[HD:P, :])

            # ---- SDPA per (head, sq-chunk), causal ----
            # sk-tile pairs run concurrently in the PE array via row groups
            # (K=64): pair element 0 in rows 0-63, element 1 in rows 64-127.
            # Each pair writes one [128, 1024] 2-bank psum tile so the exp
            # over both halves is a single ACT op. Boundary tiles
            # (o = i-4j >= 0) only compute/exp columns [128*o:512); the
            # first 128 of those get the triangular mask.
            for j in range(NCH):
                for h in range(QH):
                    qrow = (h % 2) * HD
                    chunk = slice(j * CH, (j + 1) * CH)
                    nlive = 4 * j + 4  # sk tiles 0..4j+3 are causal-live
                    offs = [max(0, (i - 4 * j)) * P for i in range(nlive)]
                    ets = []
                    with nc.named_scope("scores"):
                        for i in range(0, nlive, 2):
                            ps2 = ps_s.tile([P, 2 * CH], f32, tag="sc", name="ps_sc")
                            for u in range(2):
                                off = offs[i + u]
                                rg = slice(u * HD, (u + 1) * HD)
                                nc.tensor.matmul(
                                    ps2[:, u * CH + off : (u + 1) * CH],
                                    kt_sb[rg, (i + u) * P : (i + u + 1) * P],
                                    qt_sb[h][rg, j * CH + off : (j + 1) * CH],
                                    start=True,
                                    stop=True,
                                )
                            et2 = epool.tile([P, 2 * CH], mm_dt, tag="et", name="et")
                            with nc.named_scope("exp"):
                                if offs[i] == 0 and offs[i + 1] == 0:
                                    nc.scalar.activation(
                                        et2[:],
                                        ps2[:],
                                        mybir.ActivationFunctionType.Exp,
                                        bias=zbias[:],
                                        scale=0.125,
                                    )
                                else:
                                    for u in range(2):
                                        off = offs[i + u]
                                        nc.scalar.activation(
                                            et2[:, u * CH + off : (u + 1) * CH],
                                            ps2[:, u * CH + off : (u + 1) * CH],
                                            mybir.ActivationFunctionType.Exp,
                                            bias=zbias[:],
                                            scale=0.125,
                                        )
                            for u in range(2):
                                if i + u >= nlive - 4:  # boundary tile
                                    off = u * CH + offs[i + u]
                                    with nc.named_scope("mask"):
                                        nc.vector.tensor_mul(
                                            et2[:, off : off + P],
                                            et2[:, off : off + P],
                                            masks_sb[:],
                                        )
                            ets.append(et2)
                    pav = ps_av.tile([HD + 1, CH], f32, tag="av", name="ps_av")
                    with nc.named_scope("av"):
                        for i in range(nlive):
                            off = offs[i]
                            nc.tensor.matmul(
                                pav[:, off:],
                                vaug_sb[i][:],
                                ets[i // 2][:, (i % 2) * CH + off : (i % 2 + 1) * CH],
                                start=(i == 0),
                                stop=(i == nlive - 1),
                            )
                    # normalize: yT = avT[0:64] / avT[64]
                    with nc.named_scope("norm"):
                        recip = tpool.tile([1, CH], f32, tag="recip", name="recip")
                        nc.vector.reciprocal(recip[:], pav[HD : HD + 1, :])
                        bc = tpool.tile([HD, CH], f32, tag="bc", name="bc")
                        nc.gpsimd.partition_broadcast(bc[:], recip[:])
                        nc.vector.tensor_mul(
                            yt_sb[h // 2][qrow : qrow + HD, chunk], pav[0:HD, :], bc[:]
                        )

            # ---- output projection partial: out[sq, d] ----
            for sm in range(NSQ):
                srow = slice(sm * P, (sm + 1) * P)
                for dcJ in range(NCH):
                    dch = slice(dcJ * CH, (dcJ + 1) * CH)
                    pw = ps_a.tile([P, CH], f32, tag="proj", name="ps_wo")
                    with nc.named_scope("wo"):
                        for k in range(2):
                            nc.tensor.matmul(
                                pw[:],
                                yt_sb[k][:, srow],
                                wot_sb[k][:, dch],
                                start=(k == 0),
                                stop=(k == 1),
                            )
                    ot = tpool.tile([P, CH], f32, tag="ot", name="ot")
                    with nc.named_scope("outdma"):
                        nc.vector.tensor_copy(ot[:], pw[:])
                        odma = [nc.sync, nc.gpsimd][(sm * NCH + dcJ) % 2]
                        odma.dma_start(out_d[srow, dch], ot[:])

    nc.finalize()
    return nc


def _host_inputs(x, freqs_cos, freqs_sin, wq, wk, wv, wo):
    """Build the 8 per-core input maps (all host-side preprocessing)."""
    x = np.asarray(x, np.float32)
    cos = np.asarray(freqs_cos, np.float32)  # [S, 32]
    sin = np.asarray(freqs_sin, np.float32)
    wq = np.asarray(wq, np.float32)
    wk = np.asarray(wk, np.float32)
    wv = np.asarray(wv, np.float32)
    wo = np.asarray(wo, np.float32)

    perm = np.concatenate([np.arange(0, HD, 2), np.arange(1, HD, 2)])  # de-interleave

    xt = np.ascontiguousarray(x[0].T).astype(BF16)

    # cos64[d, t] = cos[t, d % 32]; swap64 rows 0:32 = +sin (imag-out),
    # rows 32:64 = -sin (real-out) so both tensor_mul inputs share a base
    # partition (walrus SB-SB constraint)
    cos64 = np.empty((HD, S), np.float16)
    swap64 = np.empty((HD, S), np.float16)
    for dd in range(HD):
        i = dd % 32
        cos64[dd] = cos[:, i]
        swap64[dd] = sin[:, i] if dd < 32 else -sin[:, i]

    pp = np.arange(P)[:, None]
    ff = np.arange(P)[None, :]
    masks = (pp <= ff).astype(np.float32).astype(BF16)

    ones64 = np.ones((1, HD), np.float32)
    ident = np.eye(HD, dtype=np.float32).astype(BF16)

    in_maps = []
    for c in range(NCORES):
        wq_c = wq[c * QCOLS : (c + 1) * QCOLS].reshape(QH, HD, D)[:, perm, :].reshape(
            QCOLS, D
        )
        wk_c = wk[c * HD : (c + 1) * HD][perm, :]
        wv_c = wv[c * HD : (c + 1) * HD]
        wqkvt = np.ascontiguousarray(
            np.concatenate([wq_c, wk_c, wv_c], axis=0).T
        ).astype(BF16)
        wot = np.ascontiguousarray(wo[:, c * QCOLS : (c + 1) * QCOLS].T).astype(BF16)
        in_maps.append(
            {
                "xt": xt,
                "wqkvt": wqkvt,
                "wot": wot,
                "cos64": cos64,
                "swap64": swap64,
                "masks": masks,
                "ones64": ones64,
                "ident": ident,
            }
        )
    return in_maps


def kernel(x, freqs_cos, freqs_sin, wq, wk, wv, wo):
    from concourse.bass_utils import run_bass_kernel_spmd

    if "nc" not in _CACHE:
        _CACHE["nc"] = _build()
    nc = _CACHE["nc"]
    in_maps = _host_inputs(x, freqs_cos, freqs_sin, wq, wk, wv, wo)
    res = run_bass_kernel_spmd(nc, in_maps, core_ids=list(range(NCORES)))
    out = np.zeros((S, D), np.float64)
    for r in res.results:
        out += r["out"].astype(np.float64)
    return out.astype(np.float32).reshape(1, S, D)


# revision 25
# speedup vs baseline: 8.9860x; 1.0032x over previous
"""GQA causal attention (llama3-style RoPE) on 8 TRN2 NeuronCores.

Sharding: tensor-parallel over heads. Core c gets q-heads 4c..4c+3 and
kv-head c (GQA groups intact), plus the matching row-block of wo.T.
Each core computes a full [S, D] partial of the output projection;
the host sums the 8 partials (the "all-reduce" of the row-sharded wo).

Per-core pipeline (all layouts chosen so no on-device transpose of x/q/k
is ever needed):
  qkvT[col, s]  = wqkvT.T @ xT          (weights stationary, xT streaming)
  RoPE on qT/kT (de-interleaved pair layout via host-permuted weight rows)
  sT[sk, sq]    = kT.T @ qT             (K=64)
  eT            = exp(sT/8) * causal_mask
  avT[hd+1, sq] = v_aug.T @ eT          (v augmented with a ones row ->
                                         numerator and denominator in one
                                         accumulation)
  yT            = avT[0:64] * bcast(1/avT[64])
  out[sq, d]    = yT.T @ woT            (partial; host sums over cores)
"""

import sys

for _p in ("/opt/trn_rl_repo", "/root/.axon_site/_ro/trn_rl_repo"):
    if _p not in sys.path:
        sys.path.insert(0, _p)

import numpy as np
import ml_dtypes

import concourse.bass as bass
import concourse.bacc as bacc
import concourse.mybir as mybir
import concourse.tile as tile

BF16 = ml_dtypes.bfloat16

S = 2048
D = 2048
HD = 64
NH = 32
NKV = 8
NCORES = 8
QH = NH // NCORES            # 4 local q heads
QCOLS = QH * HD              # 256
KVCOLS = 2 * HD              # 128 (k and v, one kv head)
P = 128                      # partitions
NK = D // P                  # 16 contraction tiles
NSQ = S // P                 # 16 seq tiles of 128
NCH = 4                      # seq chunks of 512
CH = 512

_CACHE = {}


def _build():
    mm_dt = mybir.dt.bfloat16
    f16 = mybir.dt.float16
    f32 = mybir.dt.float32

    nc = bacc.Bacc()
    xt_d = nc.dram_tensor("xt", [D, S], mm_dt, kind="ExternalInput")
    wqkvt_d = nc.dram_tensor("wqkvt", [D, QCOLS + KVCOLS], mm_dt, kind="ExternalInput")
    wot_d = nc.dram_tensor("wot", [QCOLS, D], mm_dt, kind="ExternalInput")
    cos_d = nc.dram_tensor("cos64", [HD, S], f16, kind="ExternalInput")
    swap_d = nc.dram_tensor("swap64", [HD, S], f16, kind="ExternalInput")
    masks_d = nc.dram_tensor("masks", [P, P], mm_dt, kind="ExternalInput")
    ones_d = nc.dram_tensor("ones64", [1, HD], f32, kind="ExternalInput")
    ident_d = nc.dram_tensor("ident", [HD, HD], mm_dt, kind="ExternalInput")
    out_d = nc.dram_tensor("out", [S, D], f32, kind="ExternalOutput")

    with tile.TileContext(nc) as tc:
        with (
            tc.tile_pool(name="const", bufs=1) as cpool,
            tc.tile_pool(name="xt", bufs=NK) as xpool,
            tc.tile_pool(name="wq", bufs=NK) as wpool,
            tc.tile_pool(name="big", bufs=1) as bigpool,
            tc.tile_pool(name="vaug", bufs=NSQ) as vpool,
            tc.tile_pool(name="et", bufs=20) as epool,
            tc.tile_pool(name="tmp", bufs=3) as tpool,
            tc.tile_pool(name="ps_a", bufs=2, space="PSUM") as ps_a,
            tc.tile_pool(name="ps_s", bufs=2, space="PSUM") as ps_s,
            tc.tile_pool(name="ps_av", bufs=2, space="PSUM") as ps_av,
        ):
            # ---- constants / weights in ----
            # small tables via SWDGE (gpsimd); bulk via the two HWDGE
            # queues (SP + ACT) in parallel
            cos_sb = cpool.tile([HD, S], f16, tag="cos")
            swap_sb = cpool.tile([HD, S], f16, tag="swap")
            masks_sb = cpool.tile([P, P], mm_dt, tag="masks")
            ones_sb = cpool.tile([1, HD], f32, tag="ones")
            ident_sb = cpool.tile([HD, HD], mm_dt, tag="ident")
            zbias = cpool.tile([P, 1], f32, tag="zbias")
            nc.gpsimd.memset(zbias[:], 0.0)
            nc.gpsimd.dma_start(cos_sb[:], cos_d[:])
            nc.gpsimd.dma_start(swap_sb[:], swap_d[:])
            nc.gpsimd.dma_start(masks_sb[:], masks_d[:])
            nc.gpsimd.dma_start(ones_sb[:], ones_d[:])
            nc.gpsimd.dma_start(ident_sb[:], ident_d[:])

            hwdge = [nc.sync, nc.scalar]
            xt_sb = []
            wq_sb = []
            for k in range(NK):
                w = wpool.tile([P, QCOLS + KVCOLS], mm_dt, tag="wq", name=f"wqkv{k}")
                hwdge[k % 2].dma_start(w[:], wqkvt_d[k * P : (k + 1) * P, :])
                wq_sb.append(w)
            for k in range(NK):
                xt_sb.append(xpool.tile([P, S], mm_dt, tag="xt", name=f"xt{k}"))
            for q in range(NCH):
                qs = slice(q * CH, (q + 1) * CH)
                for k in range(NK):
                    eng = [nc.sync, nc.scalar, nc.gpsimd][k % 3] if q == 0 else hwdge[k % 2]
                    eng.dma_start(xt_sb[k][:, qs], xt_d[k * P : (k + 1) * P, qs])

            wot_sb = []
            for k in range(2):
                t = cpool.tile([P, D], mm_dt, tag=f"wot{k}", name=f"wot{k}")
                nc.gpsimd.dma_start(t[:], wot_d[k * P : (k + 1) * P, :])
                wot_sb.append(t)

            qt_sb = [bigpool.tile([P, S], f16, tag=f"qt{m}", name=f"qt{m}") for m in range(QH)]
            kt_sb = bigpool.tile([P, S], f16, tag="kt")
            vt_sb = bigpool.tile([HD, S], mm_dt, tag="vt")
            yt_sb = [bigpool.tile([P, S], mm_dt, tag=f"yt{m}", name=f"yt{m}") for m in range(2)]

            # ---- projections: qkvT[col, sq] accumulated over d ----
            # Mtile order: kv first so SDPA can start as soon as q is ready.
            # Mtile 2: [kT; vT] | Mtile 0: q heads 0,1 | Mtile 1: q heads 2,3
            def rope(dst, ps, chunk):
                # dst[:, chunk] = RoPE(ps) for one 64-row de-interleaved head.
                # Drain psum to f16 SBUF once so the elementwise ops run in
                # the DVE 2-byte SBUF fast mode.
                qr = tpool.tile([HD, CH], f16, tag="rope_qr", name="rope_qr")
                nc.vector.tensor_copy(qr[:], ps[:])
                t2 = tpool.tile([HD, CH], f16, tag="rope_t2", name="rope_t2")
                nc.vector.tensor_mul(t2[0:32, :], qr[32:64, :], swap_sb[32:64, chunk])
                nc.vector.tensor_mul(t2[32:64, :], qr[0:32, :], swap_sb[0:32, chunk])
                nc.vector.tensor_mul(dst[:, chunk], qr[:], cos_sb[:, chunk])
                nc.vector.tensor_add(dst[:, chunk], dst[:, chunk], t2[:])

            vaug_sb = [None] * NSQ

            def vtrans(jlist):
                with nc.named_scope("vtrans"):
                    for i in jlist:
                        pt = ps_av.tile([P, HD], mm_dt, tag="av", name="ps_vt")
                        nc.tensor.transpose(
                            pt[:], vt_sb[:, i * P : (i + 1) * P], ident_sb[:]
                        )
                        va = vpool.tile([P, HD + 1], mm_dt, tag="vaug", name=f"vaug{i}")
                        nc.vector.tensor_copy(va[:, 0:HD], pt[:])
                        nc.gpsimd.memset(va[:, HD : HD + 1], 1.0)
                        vaug_sb[i] = va

            for m in (2, 0, 1):
                if m == 1:
                    vtrans(range(NSQ))
                for j in range(NCH):
                    chunk = slice(j * CH, (j + 1) * CH)
                    ps = ps_a.tile([P, CH], f32, tag="proj", name="ps_proj")
                    with nc.named_scope("proj"):
                        for k in range(NK):
                            nc.tensor.matmul(
                                ps[:],
                                wq_sb[k][:, m * P : (m + 1) * P],
                                xt_sb[k][:, chunk],
                                start=(k == 0),
                                stop=(k == NK - 1),
                            )
                    with nc.named_scope("rope"):
                        if m < 2:
                            rope(qt_sb[2 * m][0:HD, :], ps[0:HD, :], chunk)
                            rope(qt_sb[2 * m + 1][0:HD, :], ps[HD:P, :], chunk)
                            for hh in (2 * m, 2 * m + 1):
                                nc.gpsimd.dma_start(
                                    qt_sb[hh][HD:P, chunk], qt_sb[hh][0:HD, chunk]
                                )
                        else:
                            rope(kt_sb[0:HD, :], ps[0:HD, :], chunk)
                            nc.gpsimd.dma_start(kt_sb[HD:P, chunk], kt_sb[0:HD, chunk])
                            nc.vector.tensor_copy(vt_sb[:, chunk], ps[HD:P, :])

            # ---- SDPA per (head, sq-chunk), causal ----
            # sk-tile pairs run concurrently in the PE array via row groups
            # (K=64): pair element 0 in rows 0-63, element 1 in rows 64-127.
            # Each pair writes one [128, 1024] 2-bank psum tile so the exp
            # over both halves is a single ACT op. Boundary tiles
            # (o = i-4j >= 0) only compute/exp columns [128*o:512); the
            # first 128 of those get the triangular mask.
            for j in range(NCH):
                for h in range(QH):
                    qrow = (h % 2) * HD
                    chunk = slice(j * CH, (j + 1) * CH)
                    nlive = 4 * j + 4  # sk tiles 0..4j+3 are causal-live
                    offs = [max(0, (i - 4 * j)) * P for i in range(nlive)]
                    ets = []
                    with nc.named_scope("scores"):
                        for i in range(0, nlive, 2):
                            ps2 = ps_s.tile([P, 2 * CH], f32, tag="sc", name="ps_sc")
                            for u in range(2):
                                off = offs[i + u]
                                rg = slice(u * HD, (u + 1) * HD)
                                nc.tensor.matmul(
                                    ps2[:, u * CH + off : (u + 1) * CH],
                                    kt_sb[rg, (i + u) * P : (i + u + 1) * P],
                                    qt_sb[h][rg, j * CH + off : (j + 1) * CH],
                                    start=True,
                                    stop=True,
                                )
                            et2 = epool.tile([P, 2 * CH], mm_dt, tag="et", name="et")
                            with nc.named_scope("exp"):
                                if offs[i] == 0 and offs[i + 1] == 0:
                                    nc.scalar.activation(
                                        et2[:],
                                        ps2[:],
                                        mybir.ActivationFunctionType.Exp,
                                        bias=zbias[:],
                                        scale=0.125,
                                    )
                                else:
                                    for u in range(2):
                                        off = offs[i + u]
                                        nc.scalar.activation(
                                            et2[:, u * CH + off : (u + 1) * CH],
                                            ps2[:, u * CH + off : (u + 1) * CH],
                                            mybir.ActivationFunctionType.Exp,
                                            bias=zbias[:],
                                            scale=0.125,
                                        )
                            for u in range(2):
                                if i + u >= nlive - 4:  # boundary tile
                                    off = u * CH + offs[i + u]
                                    with nc.named_scope("mask"):
                                        nc.vector.tensor_mul(
                                            et2[:, off : off + P],
                                            et2[:, off : off + P],
                                            masks_sb[:],
                                        )
                            ets.append(et2)
                    pav = ps_av.tile([HD + 1, CH], f32, tag="av", name="ps_av")
                    with nc.named_scope("av"):
                        for i in range(nlive):
                            off = offs[i]
                            nc.tensor.matmul(
                                pav[:, off:],
                                vaug_sb[i][:],
                                ets[i // 2][:, (i % 2) * CH + off : (i % 2 + 1) * CH],
                                start=(i == 0),
                                stop=(i == nlive - 1),
                            )
                    # normalize: yT = avT[0:64] / avT[64]
                    with nc.named_scope("norm"):
                        recip = tpool.tile([1, CH], f32, tag="recip", name="recip")
                        nc.vector.reciprocal(recip[:], pav[HD : HD + 1, :])
                        bc = tpool.tile([HD, CH], f32, tag="bc", name="bc")
                        nc.gpsimd.partition_broadcast(bc[:], recip[:])
                        nc.vector.tensor_mul(
                            yt_sb[h // 2][qrow : qrow + HD, chunk], pav[0:HD, :], bc[:]
                        )

            # ---- output projection partial: out[sq, d] ----
            for sm in range(NSQ):
                srow = slice(sm * P, (sm + 1) * P)
                for dcJ in range(NCH):
                    dch = slice(dcJ * CH, (dcJ + 1) * CH)
                    pw = ps_a.tile([P, CH], f32, tag="proj", name="ps_wo")
                    with nc.named_scope("wo"):
                        for k in range(2):
                            nc.tensor.matmul(
                                pw[:],
                                yt_sb[k][:, srow],
                                wot_sb[k][:, dch],
                                start=(k == 0),
                                stop=(k == 1),
                            )
                    ot = tpool.tile([P, CH], f32, tag="ot", name="ot")
                    with nc.named_scope("outdma"):
                        nc.vector.tensor_copy(ot[:], pw[:])
                        if sm >= 14:
                            half = CH // 2
                            d0 = dcJ * CH
                            nc.sync.dma_start(out_d[srow, d0 : d0 + half], ot[:, 0:half])
                            nc.gpsimd.dma_start(
                                out_d[srow, d0 + half : d0 + CH], ot[:, half:CH]
                            )
                        else:
                            odma = [nc.sync, nc.gpsimd][(sm * NCH + dcJ) % 2]
                            odma.dma_start(out_d[srow, dch], ot[:])

    nc.finalize()
    return nc


def _host_inputs(x, freqs_cos, freqs_sin, wq, wk, wv, wo):
    """Build the 8 per-core input maps (all host-side preprocessing)."""
    x = np.asarray(x, np.float32)
    cos = np.asarray(freqs_cos, np.float32)  # [S, 32]
    sin = np.asarray(freqs_sin, np.float32)
    wq = np.asarray(wq, np.float32)
    wk = np.asarray(wk, np.float32)
    wv = np.asarray(wv, np.float32)
    wo = np.asarray(wo, np.float32)

    perm = np.concatenate([np.arange(0, HD, 2), np.arange(1, HD, 2)])  # de-interleave

    xt = np.ascontiguousarray(x[0].T).astype(BF16)

    # cos64[d, t] = cos[t, d % 32]; swap64 rows 0:32 = +sin (imag-out),
    # rows 32:64 = -sin (real-out) so both tensor_mul inputs share a base
    # partition (walrus SB-SB constraint)
    cos64 = np.empty((HD, S), np.float16)
    swap64 = np.empty((HD, S), np.float16)
    for dd in range(HD):
        i = dd % 32
        cos64[dd] = cos[:, i]
        swap64[dd] = sin[:, i] if dd < 32 else -sin[:, i]

    pp = np.arange(P)[:, None]
    ff = np.arange(P)[None, :]
    masks = (pp <= ff).astype(np.float32).astype(BF16)

    ones64 = np.ones((1, HD), np.float32)
    ident = np.eye(HD, dtype=np.float32).astype(BF16)

    in_maps = []
    for c in range(NCORES):
        wq_c = wq[c * QCOLS : (c + 1) * QCOLS].reshape(QH, HD, D)[:, perm, :].reshape(
            QCOLS, D
        )
        wk_c = wk[c * HD : (c + 1) * HD][perm, :]
        wv_c = wv[c * HD : (c + 1) * HD]
        wqkvt = np.ascontiguousarray(
            np.concatenate([wq_c, wk_c, wv_c], axis=0).T
        ).astype(BF16)
        wot = np.ascontiguousarray(wo[:, c * QCOLS : (c + 1) * QCOLS].T).astype(BF16)
        in_maps.append(
            {
                "xt": xt,
                "wqkvt": wqkvt,
                "wot": wot,
                "cos64": cos64,
                "swap64": swap64,
                "masks": masks,
                "ones64": ones64,
                "ident": ident,
            }
        )
    return in_maps


def kernel(x, freqs_cos, freqs_sin, wq, wk, wv, wo):
    from concourse.bass_utils import run_bass_kernel_spmd

    if "nc" not in _CACHE:
        _CACHE["nc"] = _build()
    nc = _CACHE["nc"]
    in_maps = _host_inputs(x, freqs_cos, freqs_sin, wq, wk, wv, wo)
    res = run_bass_kernel_spmd(nc, in_maps, core_ids=list(range(NCORES)))
    out = np.zeros((S, D), np.float64)
    for r in res.results:
        out += r["out"].astype(np.float64)
    return out.astype(np.float32).reshape(1, S, D)
